# revision 6
# baseline (speedup 1.0000x reference)
"""Trainium2 Bass kernel for the Cell2Cell-with-Auto module (retrieval_knn).

Strategy (8 NeuronCores, single NEFF, SPMD):
  core = 2*h + r  for head h in 0..3, r in {0,1}
  Phase A: per-core bf16 projections qT = (X@Wqk+b)^T, vT = (X@Wv_half+b)^T.
  Phase B: r-th row-half of the NxN affinity: gram via PE, exact top-k order
           statistics (rank 10 bandwidth / rank 29 threshold) via hierarchical
           Max8+match_replace on the vector engine, masked affinity -> bf16 M
           row-slab.
  AllGather (pairs) exchanges the two row-slabs of each head.
  Phase C: S = M + M^T (transpose via DMA-xbar), row sums, P = S/rowsum kept
           SBUF-resident in bf16.
  Phase D: P^t v via w <- Q w (Q = S D^-1 = P^T), using w-stationary matmuls
           with P as the moving operand; per-step transpose via DMA-xbar.
  Phase E: ReduceScatter (stride-2 groups) for the head mean, leaky ReLU,
           pair AllGather, then the row-sharded/col-split decoder
           (softplus = Ln(Exp(x)+1)) and the 12-way softmax head.

  N is padded 3000->3072 and D 2000->2048.  The 72 fake points are given
  constant coordinates 100.0, which makes them a mutually-identical far-away
  cluster: they select only each other in the knn graph, so they cannot
  pollute real rows; padded rows are discarded on the host.
"""

import sys

for _p in ("/opt/trn_rl_repo", "/root/.axon_site/_ro/trn_rl_repo"):
    if _p not in sys.path:
        sys.path.insert(0, _p)

import numpy as np
import ml_dtypes

import concourse.bass as bass
import concourse.bacc as bacc
import concourse.mybir as mybir
from concourse import tile

F32 = mybir.dt.float32
BF16 = mybir.dt.bfloat16
AF = mybir.ActivationFunctionType
OP = mybir.AluOpType

N_CORES = 8


def full_cfg(T):
    return dict(
        N=3000, NP=3072, D=2000, DP=2048, HID=256, T=T,
        DEC=2000, CLS=12, FAKE=100.0,
    )


def tiny_cfg(T):
    return dict(
        N=500, NP=512, D=200, DP=256, HID=256, T=T,
        DEC=200, CLS=12, FAKE=100.0,
    )


def derive(cfg):
    d = dict(cfg)
    d["NT"] = cfg["NP"] // 128            # row tiles of padded N
    d["NHALF"] = cfg["NP"] // 2           # rows per core in phase B
    d["NHT"] = d["NT"] // 2               # row tiles per core
    d["CH"] = min(512, cfg["NP"])         # moving chunk
    d["NCH"] = cfg["NP"] // d["CH"]
    d["DKT"] = cfg["DP"] // 128           # contraction tiles of D
    d["HT"] = cfg["HID"] // 128           # tiles of HID
    d["SELC"] = 8                         # selection chunks per row
    d["SELW"] = cfg["NP"] // 8
    d["GCH"] = 2 if d["NHT"] < 4 else 4   # allgather chunking of the M slab
    d["GTL"] = d["NHT"] // d["GCH"]       # row tiles per AG chunk
    d["RSR"] = cfg["NP"] // 4             # rows per RS shard
    d["RST"] = d["RSR"] // 128
    d["DECC"] = cfg["DEC"] // 2           # decoder cols per core
    return d


def build_program(cfg):
    c = derive(cfg)
    NP, DP, HID, T = c["NP"], c["DP"], c["HID"], c["T"]
    NT, NHALF, NHT = c["NT"], c["NHALF"], c["NHT"]
    CH, NCH, DKT, HT = c["CH"], c["NCH"], c["DKT"], c["HT"]
    SELC, SELW = c["SELC"], c["SELW"]
    GCH, GTL = c["GCH"], c["GTL"]
    RSR, RST, DECC, CLS = c["RSR"], c["RST"], c["DECC"], c["CLS"]
    NEG = -1e30

    nc = bacc.Bacc("TRN2", target_bir_lowering=False)

    # ---------------- I/O ----------------
    xt_d = nc.dram_tensor("xt", [DP, NP], BF16, kind="ExternalInput")
    wqk_d = nc.dram_tensor("wqk", [DP, HID], BF16, kind="ExternalInput")
    bqk_d = nc.dram_tensor("bqk", [128, HT], F32, kind="ExternalInput")
    wv_d = nc.dram_tensor("wv", [DP, 128], BF16, kind="ExternalInput")
    bv_d = nc.dram_tensor("bv", [128, 1], F32, kind="ExternalInput")
    wd_d = nc.dram_tensor("wd", [HID, DECC], BF16, kind="ExternalInput")
    bd_d = nc.dram_tensor("bd", [128, DECC], F32, kind="ExternalInput")
    wc_d = nc.dram_tensor("wc", [HID, CLS], BF16, kind="ExternalInput")
    bc_d = nc.dram_tensor("bc", [128, CLS], F32, kind="ExternalInput")
    eye_d = nc.dram_tensor("eye", [128, 128], BF16, kind="ExternalInput")

    recon_d = nc.dram_tensor("recon", [RSR, DECC], F32, kind="ExternalOutput")
    cls_d = nc.dram_tensor("cls", [RSR, CLS], F32, kind="ExternalOutput")

    # internal DRAM
    qt_dram = nc.dram_tensor("qt_dram", [HT, 128, NP], BF16)
    sqf_dram = nc.dram_tensor("sqf_dram", [1, NP], F32)
    m_slab = nc.dram_tensor("m_slab", [NHALF, NP], BF16)
    mg = [nc.dram_tensor(f"mg{g}", [2 * GTL * 128, NP], BF16) for g in range(GCH)]
    ctx_dram = nc.dram_tensor("ctx_dram", [NP, 128], F32)
    hid_part = nc.dram_tensor("hid_part", [RSR, 128], F32)
    hidl_dram = nc.dram_tensor("hidl_dram", [RSR, 128], F32)
    hid2_dram = nc.dram_tensor("hid2_dram", [2 * RSR, 128], F32)

    import contextlib
    with tile.TileContext(nc) as tc:
        with (
            tc.tile_pool(name="const", bufs=1) as pc,
            tc.tile_pool(name="persist", bufs=1) as pers,
            tc.tile_pool(name="psA", bufs=4, space="PSUM") as psA,
            tc.tile_pool(name="psB", bufs=2, space="PSUM") as psB,
            contextlib.ExitStack() as phase_stack,
        ):
            # ---------------- constants ----------------
            ones_f = pc.tile([128, 128], F32, tag="ones_f")
            nc.vector.memset(ones_f[:, :], 1.0)
            eye_bf = pc.tile([128, 128], BF16, tag="eye_bf")
            nc.sync.dma_start(out=eye_bf[:, :], in_=eye_d[:, :])
            pab = phase_stack.enter_context(tc.tile_pool(name="pab", bufs=1))
            paw = phase_stack.enter_context(tc.tile_pool(name="paw", bufs=2))
            wqk_sb = pab.tile([128, DKT, HID], BF16, tag="wqk_sb")
            nc.sync.dma_start(
                out=wqk_sb[:, :, :],
                in_=wqk_d.rearrange("(k p) m -> p k m", p=128))
            wv_sb = pab.tile([128, DKT, 128], BF16, tag="wv_sb")
            nc.sync.dma_start(
                out=wv_sb[:, :, :],
                in_=wv_d.rearrange("(k p) m -> p k m", p=128))
            bqk_sb = pc.tile([128, HT], F32, tag="bqk_sb")
            nc.sync.dma_start(out=bqk_sb[:, :], in_=bqk_d[:, :])
            bv_sb = pc.tile([128, 1], F32, tag="bv_sb")
            nc.sync.dma_start(out=bv_sb[:, :], in_=bv_d[:, :])

            # ---------------- persistent ----------------
            qT_bf = pab.tile([128, HT, NP], BF16, tag="qT_bf")
            vT_bf = pers.tile([128, NP], BF16, tag="vT_bf")
            sq_b = pab.tile([128, NP], F32, tag="sq_b")
            qrowsT = pab.tile([128, HT, NHALF], BF16, tag="qrowsT")
            sq_row = pab.tile([128, NHT], F32, tag="sq_row")
            rs_all = pers.tile([128, NT], F32, tag="rs_all")
            rinv_all = pers.tile([128, NT], F32, tag="rinv_all")
            w_a = pers.tile([128, NP], BF16, tag="w_a")
            w_b = pers.tile([128, NP], BF16, tag="w_b")
            wT_bf = pers.tile([128, NP], BF16, tag="wT_bf")

            # ================ PHASE A: projections ================
            for ccI in range(NCH):
                cs = slice(ccI * CH, (ccI + 1) * CH)
                xt_ch = paw.tile([128, DKT, CH], BF16, tag="xt_ch")
                nc.sync.dma_start(
                    out=xt_ch[:, :, :],
                    in_=xt_d[:, cs].rearrange("(k p) f -> p k f", p=128))
                for m in range(HT):
                    ps_q = psA.tile([128, CH], F32, tag="mmps")
                    for k in range(DKT):
                        nc.tensor.matmul(
                            ps_q[:, :],
                            wqk_sb[:, k, m * 128:(m + 1) * 128],
                            xt_ch[:, k, :],
                            start=(k == 0), stop=(k == DKT - 1))
                    nc.vector.tensor_scalar_add(
                        qT_bf[:, m, cs], ps_q[:, :], bqk_sb[:, m:m + 1])
                ps_v = psA.tile([128, CH], F32, tag="mmps")
                for k in range(DKT):
                    nc.tensor.matmul(
                        ps_v[:, :], wv_sb[:, k, :], xt_ch[:, k, :],
                        start=(k == 0), stop=(k == DKT - 1))
                nc.vector.tensor_scalar_add(
                    vT_bf[:, cs], ps_v[:, :], bv_sb[:, 0:1])
                # squared q, then broadcast column sums (fp32 for accuracy)
                ps_s = psB.tile([128, CH], F32, tag="auxps")
                for m in range(HT):
                    qsq = paw.tile([128, CH], F32, tag="qsq")
                    nc.scalar.activation(qsq[:, :], qT_bf[:, m, cs], AF.Square)
                    nc.tensor.matmul(
                        ps_s[:, :], ones_f[:, :], qsq[:, :],
                        start=(m == 0), stop=(m == HT - 1))
                nc.scalar.copy(sq_b[:, cs], ps_s[:, :])

            # qT + sq roundtrip through DRAM for the dynamic row-half slice
            for m in range(HT):
                nc.sync.dma_start(out=qt_dram[m, :, :], in_=qT_bf[:, m, :])
            nc.sync.dma_start(out=sqf_dram[0:1, :], in_=sq_b[0:1, :])
            rid = nc.gpsimd.partition_id()
            off = nc.gpsimd.snap((rid % 2) * NHALF)
            for m in range(HT):
                nc.gpsimd.dma_start(
                    out=qrowsT[:, m, :], in_=qt_dram[m, :, bass.ds(off, NHALF)])
            sqr_flat = pab.tile([1, NHALF], F32, tag="sqr_flat")
            nc.gpsimd.dma_start(
                out=sqr_flat[0:1, :], in_=sqf_dram[0:1, bass.ds(off, NHALF)])
            for t in range(NHT):
                ps_t = psB.tile([128, 1], F32, tag="auxps")
                nc.tensor.matmul(
                    ps_t[:, :], sqr_flat[0:1, t * 128:(t + 1) * 128],
                    ones_f[0:1, 0:1], start=True, stop=True)
                nc.scalar.copy(sq_row[:, t:t + 1], ps_t[:, :])

            # ================ PHASE B: affinity row-slab ================
            selp = phase_stack.enter_context(tc.tile_pool(name="selp", bufs=1))
            for t in range(NHT):
                cp = selp.tile([128, NP], F32, tag="cp")
                for ccI in range(NCH):
                    cs = slice(ccI * CH, (ccI + 1) * CH)
                    ps_g = psA.tile([128, CH], F32, tag="mmps")
                    for k in range(HT):
                        nc.tensor.matmul(
                            ps_g[:, :],
                            qrowsT[:, k, t * 128:(t + 1) * 128],
                            qT_bf[:, k, cs],
                            start=(k == 0), stop=(k == HT - 1))
                    # c' = 2G - sq_j
                    nc.vector.scalar_tensor_tensor(
                        cp[:, cs], ps_g[:, :], 2.0, sq_b[:, cs],
                        op0=OP.mult, op1=OP.subtract)
                # ---- selection: top-16 of each of SELC chunks, then top-32 ----
                sel = selp.tile([128, NP], F32, tag="sel")
                nc.scalar.copy(sel[:, :], cp[:, :])
                cand = selp.tile([128, 16 * SELC], F32, tag="cand")
                for chI in range(SELC):
                    ss = sel[:, chI * SELW:(chI + 1) * SELW]
                    c0 = chI * 16
                    nc.vector.max(cand[:, c0:c0 + 8], ss)
                    nc.vector.match_replace(ss, cand[:, c0:c0 + 8], ss, NEG)
                    nc.vector.max(cand[:, c0 + 8:c0 + 16], ss)
                m8 = selp.tile([128, 32], F32, tag="m8")
                for r in range(4):
                    nc.vector.max(m8[:, r * 8:(r + 1) * 8], cand[:, :])
                    if r < 3:
                        nc.vector.match_replace(
                            cand[:, :], m8[:, r * 8:(r + 1) * 8], cand[:, :], NEG)
                # ---- bandwidth r = 1/(d2_10 + 1e-10) ----
                s2 = selp.tile([128, 1], F32, tag="s2")
                nc.vector.tensor_tensor(
                    s2[:, :], sq_row[:, t:t + 1], m8[:, 10:11], op=OP.subtract)
                nc.vector.tensor_scalar_add(s2[:, :], s2[:, :], 1e-10)
                rr = selp.tile([128, 1], F32, tag="rr")
                nc.vector.reciprocal(rr[:, :], s2[:, :])
                nrr = selp.tile([128, 1], F32, tag="nrr")
                nc.vector.tensor_scalar_mul(nrr[:, :], rr[:, :], -1.0)
                bee = selp.tile([128, 1], F32, tag="bee")
                nc.vector.tensor_scalar_mul(bee[:, :], rr[:, :], -1e-10)
                # ---- affinity + mask ----
                d2c = selp.tile([128, NP], F32, tag="sel")
                nc.vector.tensor_scalar(
                    d2c[:, :], cp[:, :], sq_row[:, t:t + 1], -1.0,
                    op0=OP.subtract, op1=OP.mult)
                aff = selp.tile([128, NP], BF16, tag="aff")
                nc.scalar.activation(
                    aff[:, :], d2c[:, :], AF.Exp,
                    bias=bee[:, 0:1], scale=nrr[:, 0:1])
                m_bf = selp.tile([128, NP], BF16, tag="sel")
                nc.vector.scalar_tensor_tensor(
                    m_bf[:, :], cp[:, :], m8[:, 29:30], aff[:, :],
                    op0=OP.is_ge, op1=OP.mult)
                nc.sync.dma_start(
                    out=m_slab[t * 128:(t + 1) * 128, :], in_=m_bf[:, :])

            phase_stack.close()
            pcd = phase_stack.enter_context(tc.tile_pool(name="pcd", bufs=1))
            pcw = phase_stack.enter_context(tc.tile_pool(name="pcw", bufs=2))
            p_big = pcd.tile([128, NT * NP], BF16, tag="p_big")
            # ================ AllGather M (chunked) ================
            for g in range(GCH):
                nc.gpsimd.collective_compute(
                    "AllGather", OP.bypass,
                    replica_groups=[[0, 1], [2, 3], [4, 5], [6, 7]],
                    ins=[m_slab[g * GTL * 128:(g + 1) * GTL * 128, :].opt()],
                    outs=[mg[g].ap().opt()])

            # ================ PHASE C: S = M + M^T, P = S/rowsum ===========
            for j in range(NT):
                rank, loc = divmod(j, NHT)
                g, wi = divmod(loc, GTL)
                pj = p_big[:, j * NP:(j + 1) * NP]
                src0 = (rank * GTL + wi) * 128
                nc.sync.dma_start(out=pj, in_=mg[g][src0:src0 + 128, :])
                mcolT = pcd.tile([128, NP], BF16, tag="mcolT")
                for g2 in range(GCH):
                    for rk in range(2):
                        dst = rk * NHALF + g2 * GTL * 128
                        nc.sync.dma_start_transpose(
                            mcolT[:, dst:dst + GTL * 128],
                            mg[g2][rk * GTL * 128:(rk + 1) * GTL * 128,
                                   j * 128:(j + 1) * 128])
                nc.vector.scalar_tensor_tensor(
                    pj, pj, 1.0, mcolT[:, :],
                    op0=OP.mult, op1=OP.add, accum_out=rs_all[:, j:j + 1])
                nc.vector.reciprocal(rinv_all[:, j:j + 1], rs_all[:, j:j + 1])
                nc.vector.tensor_scalar_mul(pj, pj, rinv_all[:, j:j + 1])

            # ================ PHASE D: power iterations ================
            for k in range(NT):
                nc.sync.dma_start_transpose(
                    w_a[:, k * 128:(k + 1) * 128],
                    vT_bf[:, k * 128:(k + 1) * 128])
                nc.vector.tensor_scalar_mul(
                    w_a[:, k * 128:(k + 1) * 128],
                    w_a[:, k * 128:(k + 1) * 128], rs_all[:, k:k + 1])
            for s in range(T):
                win = w_a if s % 2 == 0 else w_b
                wout = w_b if s % 2 == 0 else w_a
                for ccI in range(NCH):
                    ps_w = psA.tile([128, CH], F32, tag="mmps")
                    for k in range(NT):
                        nc.tensor.matmul(
                            ps_w[:, :],
                            win[:, k * 128:(k + 1) * 128],
                            p_big[:, k * NP + ccI * CH:k * NP + (ccI + 1) * CH],
                            start=(k == 0), stop=(k == NT - 1))
                    nc.scalar.copy(wT_bf[:, ccI * CH:(ccI + 1) * CH], ps_w[:, :])
                for k in range(NT):
                    nc.sync.dma_start_transpose(
                        wout[:, k * 128:(k + 1) * 128],
                        wT_bf[:, k * 128:(k + 1) * 128])
            wfin = w_a if T % 2 == 0 else w_b
            for k in range(NT):
                ctx_sb = pcw.tile([128, 128], F32, tag="ctx_sb")
                nc.vector.tensor_scalar(
                    ctx_sb[:, :], wfin[:, k * 128:(k + 1) * 128],
                    rinv_all[:, k:k + 1], 0.25, op0=OP.mult, op1=OP.mult)
                nc.sync.dma_start(
                    out=ctx_dram[k * 128:(k + 1) * 128, :], in_=ctx_sb[:, :])

            phase_stack.close()
            pe = phase_stack.enter_context(tc.tile_pool(name="pe", bufs=1))
            pew = phase_stack.enter_context(tc.tile_pool(name="pew", bufs=2))
            # ================ PHASE E: mean + leaky + decoder =============
            nc.gpsimd.collective_compute(
                "ReduceScatter", OP.add,
                replica_groups=[[0, 2, 4, 6], [1, 3, 5, 7]],
                ins=[ctx_dram.ap().opt()], outs=[hid_part.ap().opt()])
            for b in range(RST):
                hb = pew.tile([128, 128], F32, tag="hb")
                nc.sync.dma_start(
                    out=hb[:, :], in_=hid_part[b * 128:(b + 1) * 128, :])
                nc.vector.scalar_tensor_tensor(
                    hb[:, :], hb[:, :], 0.01, hb[:, :],
                    op0=OP.mult, op1=OP.max)
                nc.sync.dma_start(
                    out=hidl_dram[b * 128:(b + 1) * 128, :], in_=hb[:, :])
            nc.gpsimd.collective_compute(
                "AllGather", OP.bypass,
                replica_groups=[[0, 1], [2, 3], [4, 5], [6, 7]],
                ins=[hidl_dram.ap().opt()], outs=[hid2_dram.ap().opt()])

            hidT = pe.tile([128, HT, RSR], BF16, tag="hidT")
            for b in range(2 * RST):
                kk, bb = divmod(b, RST)
                hb2 = pew.tile([128, 128], F32, tag="hb2")
                nc.sync.dma_start(
                    out=hb2[:, :], in_=hid2_dram[b * 128:(b + 1) * 128, :])
                hb2f = pew.tile([128, 128], BF16, tag="hb2f")
                nc.vector.tensor_copy(hb2f[:, :], hb2[:, :])
                ps_h = psB.tile([128, 128], BF16, tag="auxps")
                nc.tensor.transpose(ps_h[:, :], hb2f[:, :], eye_bf[:, :])
                nc.scalar.copy(hidT[:, kk, bb * 128:(bb + 1) * 128], ps_h[:, :])

            wd_sb = pe.tile([128, HT, DECC], BF16, tag="wd_sb")
            nc.sync.dma_start(
                out=wd_sb[:, :, :],
                in_=wd_d.rearrange("(k p) n -> p k n", p=128))
            wc_sb = pe.tile([128, HT, CLS], BF16, tag="wc_sb")
            nc.sync.dma_start(
                out=wc_sb[:, :, :],
                in_=wc_d.rearrange("(k p) n -> p k n", p=128))
            bd_sb = pe.tile([128, DECC], F32, tag="bd_sb")
            nc.sync.dma_start(out=bd_sb[:, :], in_=bd_d[:, :])
            bc_sb = pe.tile([128, CLS], F32, tag="bc_sb")
            nc.sync.dma_start(out=bc_sb[:, :], in_=bc_d[:, :])

            dec_chunks = []
            c0 = 0
            while c0 < DECC:
                cw = min(512, DECC - c0)
                dec_chunks.append((c0, cw))
                c0 += cw
            for i in range(RST):
                isl = slice(i * 128, (i + 1) * 128)
                for (c0, cw) in dec_chunks:
                    ps_r = psA.tile([128, 512], F32, tag="mmps")
                    for k in range(HT):
                        nc.tensor.matmul(
                            ps_r[:, 0:cw],
                            hidT[:, k, isl],
                            wd_sb[:, k, c0:c0 + cw],
                            start=(k == 0), stop=(k == HT - 1))
                    tmp = pew.tile([128, 512], F32, tag="dtmp")
                    nc.vector.scalar_tensor_tensor(
                        tmp[:, 0:cw], ps_r[:, 0:cw], 1.0, bd_sb[:, c0:c0 + cw],
                        op0=OP.mult, op1=OP.add)
                    tmp2 = pew.tile([128, 512], F32, tag="dtmp2")
                    nc.scalar.activation(tmp2[:, 0:cw], tmp[:, 0:cw], AF.Exp)
                    rec = pew.tile([128, 512], F32, tag="drec")
                    nc.scalar.activation(
                        rec[:, 0:cw], tmp2[:, 0:cw], AF.Ln, bias=1.0)
                    nc.sync.dma_start(
                        out=recon_d[isl, c0:c0 + cw], in_=rec[:, 0:cw])
                # classifier head
                ps_c = psB.tile([128, CLS], F32, tag="auxps")
                for k in range(HT):
                    nc.tensor.matmul(
                        ps_c[:, :], hidT[:, k, isl], wc_sb[:, k, :],
                        start=(k == 0), stop=(k == HT - 1))
                lg = pew.tile([128, CLS], F32, tag="lg")
                nc.vector.scalar_tensor_tensor(
                    lg[:, :], ps_c[:, :], 1.0, bc_sb[:, :],
                    op0=OP.mult, op1=OP.add)
                mx = pew.tile([128, 1], F32, tag="mx")
                nc.vector.tensor_reduce(
                    mx[:, :], lg[:, :], axis=mybir.AxisListType.X, op=OP.max)
                nmx = pew.tile([128, 1], F32, tag="nmx")
                nc.vector.tensor_scalar_mul(nmx[:, :], mx[:, :], -1.0)
                eo = pew.tile([128, CLS], F32, tag="eo")
                se = pew.tile([128, 1], F32, tag="se")
                nc.scalar.activation(
                    eo[:, :], lg[:, :], AF.Exp, bias=nmx[:, 0:1],
                    accum_out=se[:, :])
                rse = pew.tile([128, 1], F32, tag="rse")
                nc.vector.reciprocal(rse[:, :], se[:, :])
                cls_sb = pew.tile([128, CLS], F32, tag="cls_sb")
                nc.vector.tensor_scalar_mul(cls_sb[:, :], eo[:, :], rse[:, 0:1])
                nc.sync.dma_start(out=cls_d[isl, :], in_=cls_sb[:, :])

    nc.compile()
    return nc


def make_in_maps(cfg, inputs):
    c = derive(cfg)
    N, NP, D, DP, HID = c["N"], c["NP"], c["D"], c["DP"], c["HID"]
    DEC, CLS, DECC, HT = c["DEC"], c["CLS"], c["DECC"], c["HT"]
    bf = ml_dtypes.bfloat16

    X = np.asarray(inputs["input_tensor"], np.float32)
    Wqk = np.asarray(inputs["Wqk"], np.float32)
    bqk = np.asarray(inputs["bqk"], np.float32)
    Wv = np.asarray(inputs["Wv"], np.float32)
    bv = np.asarray(inputs["bv"], np.float32)
    Wd = np.asarray(inputs["Wd"], np.float32)
    bd = np.asarray(inputs["bd"], np.float32)
    Wc = np.asarray(inputs["Wc"], np.float32)
    bc = np.asarray(inputs["bc"], np.float32)

    xt = np.zeros((DP, NP), np.float32)
    xt[:D, :N] = X.T
    # Fake points: a far-away loose cluster (mutually separated by O(1)
    # q-space distances, ~sqrt(1e6) from every real point) so each fake
    # point's knn is contained in the fake cluster and its affinity to any
    # real point underflows to exactly 0.
    rng = np.random.default_rng(12345)
    xt[:D, N:] = cfg["FAKE"] + 5.0 * rng.standard_normal((D, NP - N))
    xt_bf = xt.astype(bf)
    eye_bf = np.eye(128, dtype=bf)

    in_maps = []
    for core in range(N_CORES):
        h, ccol = divmod(core, 2)
        wqk_p = np.zeros((DP, HID), np.float32)
        wqk_p[:D] = Wqk[h]
        wv_p = np.zeros((DP, 128), np.float32)
        wv_p[:D] = Wv[h][:, ccol * 128:(ccol + 1) * 128]
        in_maps.append({
            "xt": xt_bf,
            "wqk": wqk_p.astype(bf),
            "bqk": np.ascontiguousarray(bqk[h].reshape(HT, 128).T),
            "wv": wv_p.astype(bf),
            "bv": np.ascontiguousarray(
                bv[h][ccol * 128:(ccol + 1) * 128].reshape(128, 1)),
            "wd": Wd[:, ccol * DECC:(ccol + 1) * DECC].astype(bf),
            "bd": np.ascontiguousarray(np.broadcast_to(
                bd[ccol * DECC:(ccol + 1) * DECC], (128, DECC))),
            "wc": Wc.astype(bf),
            "bc": np.ascontiguousarray(np.broadcast_to(bc, (128, CLS))),
            "eye": eye_bf,
        })
    return in_maps


def assemble(cfg, results):
    c = derive(cfg)
    N, DEC, CLS, DECC, RSR = c["N"], c["DEC"], c["CLS"], c["DECC"], c["RSR"]
    recon = np.zeros((c["NP"], DEC), np.float32)
    cls = np.zeros((c["NP"], CLS), np.float32)
    for core in range(N_CORES):
        h, ccol = divmod(core, 2)
        rows = slice(h * RSR, (h + 1) * RSR)
        recon[rows, ccol * DECC:(ccol + 1) * DECC] = results[core]["recon"]
        if ccol == 0:
            cls[rows] = results[core]["cls"]
    return recon[:N], cls[:N]


_prog_cache = {}


def kernel(**inputs):
    from concourse.bass_utils import run_bass_kernel_spmd
    t = int(np.asarray(inputs["t"]))
    cfg = full_cfg(t)
    key = ("full", t)
    if key not in _prog_cache:
        _prog_cache[key] = build_program(cfg)
    nc = _prog_cache[key]
    in_maps = make_in_maps(cfg, inputs)
    res = run_bass_kernel_spmd(nc, in_maps, core_ids=list(range(N_CORES)))
    return assemble(cfg, res.results)
